# revision 14
# baseline (speedup 1.0000x reference)
"""Causal self-attention on 8 Trainium2 NeuronCores.

Problem: B=4, T=2048, C=1024, H=16, DH=64.
  qkv = x @ w_qkv.T ; causal softmax attention per head ; y = attnout @ w_out.T

Sharding: 8 cores = 4 batches x 2 query-subsets. Each core computes the full
QKV projection for its batch (duplicated within the pair -> no collectives),
then attention for a load-balanced set of query blocks (all 16 heads), then
the output projection for its own query rows against the full w_out. No
cross-core communication anywhere.

Query-block balance (causal): global 256-row q-tiles are paired (i, 7-i) so
both parity programs do identical total key-tile work (36 k-tiles each):
  parity 0 -> q256 tiles [0, 1, 6, 7], parity 1 -> [2, 3, 4, 5].

Everything runs in "transposed space": Q^T/K^T are produced head-pair-stacked
[128=2x64 dh rows, T], scores are computed as S^T (keys on PSUM partitions,
two heads concurrently via PE row-tiling), softmax normalization over keys
uses a DVE accumulator + ones-matmul partition reduction, PV produces
attnout^T directly (two heads via PE column-tiling), and the output
projection consumes attnout^T as its stationary operand — no transposes in
any inner loop.
"""

import threading

import numpy as np

B, T, C = 4, 2048, 1024
H = 16
DH = C // H
P = 128
TL = T // 2          # query rows per core
NPAIR = H // 2       # 8 head-pairs
NCT = C // P         # 8 c-tiles
QT_TILE = 256        # q columns per attention tile
NQT = TL // QT_TILE  # 4 local q-tiles
NEG = -1.0e9

# local q256-tile -> global q256-tile, per parity
QMAP256 = [[0, 1, 6, 7], [2, 3, 4, 5]]
# local 512-row blocks of the q-half -> global 512-row block (for Q projection)
QMAP512 = [[0, 3], [1, 2]]

_cache = {}


def _build_program(parity: int):
    import concourse.mybir as mybir
    import concourse.tile as tile
    from concourse import bacc
    from concourse.masks import make_identity
    from concourse.tile import add_dep_helper

    f32 = mybir.dt.float32
    bf16 = mybir.dt.bfloat16

    nc = bacc.Bacc("TRN2", target_bir_lowering=False, debug=False)
    x = nc.dram_tensor("x", [T, C], f32, kind="ExternalInput").ap()
    w_qkv = nc.dram_tensor("w_qkv", [3 * C, C], f32, kind="ExternalInput").ap()
    w_out = nc.dram_tensor("w_out", [C, C], f32, kind="ExternalInput").ap()
    y = nc.dram_tensor("y", [TL, C], f32, kind="ExternalOutput").ap()

    g256 = QMAP256[parity]
    g512 = QMAP512[parity]

    with tile.TileContext(nc) as tc:
        with (
            tc.tile_pool(name="res", bufs=1) as res,          # long-lived residents
            tc.tile_pool(name="stage", bufs=2) as stage,       # load/cast staging
            tc.tile_pool(name="wtile", bufs=2) as wtile,       # transposed weight tiles
            tc.tile_pool(name="work", bufs=3) as work,         # misc working tiles
            tc.tile_pool(name="acc", bufs=2) as acc,           # softmax accumulators
            tc.tile_pool(name="attn", bufs=2) as attnp,        # attnout^T per q-tile
            tc.tile_pool(name="yout", bufs=2) as yout,
        ):
            ident = res.tile([P, P], bf16)
            make_identity(nc, ident)
            ones128 = res.tile([P, P], bf16)
            nc.vector.memset(ones128, 1.0)

            # ---- causal mask tiles for the two diagonal k-tiles of a q-tile.
            # S^T layout: [k row, q col]; allowed iff global_k <= global_q.
            # diag tile 0 (k rows q0..q0+127):   keep iff col >= row
            # diag tile 1 (k rows q0+128..+255): keep iff col >= row + 128
            mask0 = res.tile([P, QT_TILE], f32)
            mask1 = res.tile([P, QT_TILE], f32)
            for m, base in ((mask0, 0), (mask1, -P)):
                nc.gpsimd.memset(m, 0.0)
                nc.gpsimd.affine_select(
                    out=m, in_=m, compare_op=mybir.AluOpType.is_ge,
                    fill=NEG, base=base, pattern=[[1, QT_TILE]],
                    channel_multiplier=-1,
                )

            # ---- residents
            xT = res.tile([P, NCT, T], bf16)            # x^T   4 MB
            kT = res.tile([P, NPAIR, T], bf16)          # K^T   4 MB
            qT = res.tile([P, NPAIR, TL], bf16)         # Q^T   2 MB
            v = res.tile([P, T // P, C], bf16)          # V     4 MB
            wvT = res.tile([P, NCT, C], bf16)           # w_v^T 2 MB
            woT = res.tile([P, NCT, C], bf16)           # w_out^T 2 MB

            with (
                tc.tile_pool(name="pst", bufs=2, space="PSUM") as pst,
                tc.tile_pool(name="psqkv", bufs=4, space="PSUM") as psqkv,
            ):
                # ================= Phase 0: x^T =================
                for tt in range(T // P):
                    xb = stage.tile([P, C], bf16, tag="ldb")
                    nc.gpsimd.dma_start(out=xb, in_=x[tt * P:(tt + 1) * P, :])
                    for ct in range(NCT):
                        pt = pst.tile([P, P], bf16, tag="pt")
                        nc.tensor.transpose(pt, xb[:, ct * P:(ct + 1) * P], ident)
                        nc.any.tensor_copy(
                            out=xT[:, ct, tt * P:(tt + 1) * P], in_=pt)

                # ======= Phase 1: weight transposes + Q/K/V projections =======
                # w_qkv rows: [0:1024) Q, [1024:2048) K, [2048:3072) V.
                def transpose_into(src, r0, dst, dst_col):
                    """Transpose src rows [r0, r0+128) x all C cols into
                    dst[:, ct, dst_col:dst_col+128]."""
                    wb = stage.tile([P, C], bf16, tag="ldb")
                    nc.gpsimd.dma_start(out=wb, in_=src[r0:r0 + P, :])
                    for ct in range(NCT):
                        pt = pst.tile([P, P], bf16, tag="pt")
                        nc.tensor.transpose(pt, wb[:, ct * P:(ct + 1) * P], ident)
                        nc.any.tensor_copy(
                            out=dst[:, ct, dst_col:dst_col + P], in_=pt)

                for fb in range(8):
                    transpose_into(w_qkv, (16 + fb) * P, wvT, fb * P)
                for fb in range(8):
                    transpose_into(w_out, fb * P, woT, fb * P)

                # Q/K: per f-tile, transpose weights then project against x^T
                for fb in range(16):                     # 0..7 Q, 8..15 K
                    wqk = wtile.tile([P, NCT, P], bf16, tag="wqk")
                    wb = stage.tile([P, C], bf16, tag="ldb")
                    nc.gpsimd.dma_start(out=wb, in_=w_qkv[fb * P:(fb + 1) * P, :])
                    for ct in range(NCT):
                        pt = pst.tile([P, P], bf16, tag="pt")
                        nc.tensor.transpose(pt, wb[:, ct * P:(ct + 1) * P], ident)
                        nc.any.tensor_copy(out=wqk[:, ct, :], in_=pt)

                    if fb < 8:
                        # Q: only the local query half, in local column order
                        for u in range(2):
                            ps = psqkv.tile([P, 512], f32, tag="psqkv")
                            t0 = g512[u] * 512
                            for ct in range(NCT):
                                nc.tensor.matmul(
                                    ps, wqk[:, ct, :], xT[:, ct, t0:t0 + 512],
                                    start=(ct == 0), stop=(ct == NCT - 1),
                                )
                            nc.any.tensor_copy(
                                out=qT[:, fb, u * 512:(u + 1) * 512], in_=ps)
                    else:
                        pr = fb - 8
                        for u in range(4):
                            ps = psqkv.tile([P, 512], f32, tag="psqkv")
                            for ct in range(NCT):
                                nc.tensor.matmul(
                                    ps, wqk[:, ct, :],
                                    xT[:, ct, u * 512:(u + 1) * 512],
                                    start=(ct == 0), stop=(ct == NCT - 1),
                                )
                            nc.any.tensor_copy(
                                out=kT[:, pr, u * 512:(u + 1) * 512], in_=ps)

                # ================= Phase 2: V projection =================
                for tt in range(T // P):
                    for fo in range(2):
                        ps = psqkv.tile([P, 512], f32, tag="psqkv")
                        for ct in range(NCT):
                            nc.tensor.matmul(
                                ps, xT[:, ct, tt * P:(tt + 1) * P],
                                wvT[:, ct, fo * 512:(fo + 1) * 512],
                                start=(ct == 0), stop=(ct == NCT - 1),
                            )
                        nc.any.tensor_copy(
                            out=v[:, tt, fo * 512:(fo + 1) * 512], in_=ps)

            # ================= Phase 3: attention + out-proj =================
            with (
                tc.tile_pool(name="pss", bufs=4, space="PSUM") as pss,
                tc.tile_pool(name="pso", bufs=2, space="PSUM") as pso,
                tc.tile_pool(name="psy", bufs=2, space="PSUM") as psy,
            ):
                scale = 1.0 / float(np.sqrt(DH))
                for j in range(NQT):
                    G = g256[j]
                    nk = 2 * (G + 1)    # k-tiles (keys 0 .. 256*(G+1))
                    attnT = attnp.tile([P, NPAIR, QT_TILE], bf16, tag="attnT")
                    for p in range(NPAIR):
                        # separate banks per head; B sits at partitions 64:128
                        # so the PE runs both PV matmuls on disjoint col groups
                        poA_t = pso.tile([P, QT_TILE], f32, tag="po", name="poA")
                        poB_t = pso.tile([P, QT_TILE], f32, tag="po", name="poB")
                        poA = poA_t[0:64]
                        poB = poB_t[64:128]
                        dA = acc.tile([P, QT_TILE], f32, tag="dA")
                        dB = acc.tile([P, QT_TILE], f32, tag="dB")
                        qA = qT[0:64, p, j * QT_TILE:(j + 1) * QT_TILE]
                        qB = qT[64:128, p, j * QT_TILE:(j + 1) * QT_TILE]
                        for k in range(nk):
                            sA = pss.tile([P, QT_TILE], f32, tag="s")
                            sB = pss.tile([P, QT_TILE], f32, tag="s")
                            ks = slice(k * P, (k + 1) * P)
                            nc.tensor.matmul(sA, kT[0:64, p, ks], qA)
                            nc.tensor.matmul(sB, kT[64:128, p, ks], qB)
                            if k >= nk - 2:  # diagonal tiles
                                m = mask0 if k == nk - 2 else mask1
                                nc.vector.tensor_add(out=sA, in0=sA, in1=m)
                                nc.vector.tensor_add(out=sB, in0=sB, in1=m)
                            pA = work.tile([P, QT_TILE], bf16, tag="pA")
                            pB = work.tile([P, QT_TILE], bf16, tag="pB")
                            nc.scalar.activation(
                                pA, sA, mybir.ActivationFunctionType.Exp,
                                scale=scale)
                            nc.scalar.activation(
                                pB, sB, mybir.ActivationFunctionType.Exp,
                                scale=scale)
                            if k == 0:
                                nc.vector.tensor_copy(out=dA, in_=pA)
                                nc.vector.tensor_copy(out=dB, in_=pB)
                            else:
                                nc.vector.tensor_add(out=dA, in0=dA, in1=pA)
                                nc.vector.tensor_add(out=dB, in0=dB, in1=pB)
                            nc.tensor.matmul(
                                poA, v[:, k, p * P:p * P + 64], pA,
                                start=(k == 0), stop=(k == nk - 1))
                            nc.tensor.matmul(
                                poB, v[:, k, p * P + 64:(p + 1) * P], pB,
                                start=(k == 0), stop=(k == nk - 1))
                        # denominators: the all-ones stationary reduces over
                        # key partitions AND replicates the column sums to
                        # every output partition in one matmul.
                        dAc = work.tile([P, QT_TILE], bf16, tag="dAc")
                        dBc = work.tile([P, QT_TILE], bf16, tag="dBc")
                        nc.vector.tensor_copy(out=dAc, in_=dA)
                        nc.vector.tensor_copy(out=dBc, in_=dB)
                        pDA = pss.tile([P, QT_TILE], f32, tag="s")
                        pDB = pss.tile([P, QT_TILE], f32, tag="s")
                        nc.tensor.matmul(pDA, ones128, dAc)
                        nc.tensor.matmul(pDB, ones128, dBc)
                        rD = work.tile([P, QT_TILE], f32, tag="rD")
                        nc.vector.reciprocal(rD[0:64], pDA[0:64])
                        nc.vector.reciprocal(rD[64:128], pDB[64:128])
                        # normalize into attnout^T (cast to bf16 on write)
                        nc.vector.tensor_mul(
                            out=attnT[0:64, p, :], in0=poA, in1=rD[0:64])
                        nc.vector.tensor_mul(
                            out=attnT[64:128, p, :], in0=poB, in1=rD[64:128])
                    # ---- output projection for this q-tile
                    for sub in range(2):
                        qs = slice(sub * P, (sub + 1) * P)
                        for fo in range(2):
                            ps = psy.tile([P, 512], f32, tag="psy")
                            for p in range(NPAIR):
                                nc.tensor.matmul(
                                    ps, attnT[:, p, qs],
                                    woT[:, p, fo * 512:(fo + 1) * 512],
                                    start=(p == 0), stop=(p == NPAIR - 1),
                                )
                            ysb = yout.tile([P, 512], f32, tag="ysb")
                            nc.any.tensor_copy(out=ysb, in_=ps)
                            nc.sync.dma_start(
                                out=y[j * QT_TILE + sub * P:
                                      j * QT_TILE + (sub + 1) * P,
                                      fo * 512:(fo + 1) * 512],
                                in_=ysb)

    nc.compile()
    return nc


def _get_program(parity: int):
    if parity not in _cache:
        _cache[parity] = _build_program(parity)
    return _cache[parity]


def _run_group(nc, in_maps, devices, out_holder, idx):
    """shard_map the program over `devices`, one in_map per device."""
    import jax
    from jax.sharding import Mesh, PartitionSpec
    from jax.experimental.shard_map import shard_map
    import concourse.mybir as mybir
    from concourse.bass2jax import (
        _bass_exec_p, install_neuronx_cc_hook, partition_id_tensor)

    install_neuronx_cc_hook()

    partition_name = (
        nc.partition_id_tensor.name if nc.partition_id_tensor else None)
    in_names, out_names, out_avals, zero_outs = [], [], [], []
    for alloc in nc.m.functions[0].allocations:
        if not isinstance(alloc, mybir.MemoryLocationSet):
            continue
        name = alloc.memorylocations[0].name
        if alloc.kind == "ExternalInput":
            if name != partition_name:
                in_names.append(name)
        elif alloc.kind == "ExternalOutput":
            out_names.append(name)
            shape = tuple(alloc.tensor_shape)
            dtype = mybir.dt.np(alloc.dtype)
            out_avals.append(jax.core.ShapedArray(shape, dtype))
            zero_outs.append(np.zeros(shape, dtype))
    n_params = len(in_names)
    n_outs = len(out_avals)
    all_names = in_names + out_names
    if partition_name is not None:
        all_names.append(partition_name)
    donate = tuple(range(n_params, n_params + n_outs))

    def _body(*args):
        operands = list(args)
        if partition_name is not None:
            operands.append(partition_id_tensor())
        outs = _bass_exec_p.bind(
            *operands,
            out_avals=tuple(out_avals),
            in_names=tuple(all_names),
            out_names=tuple(out_names),
            lowering_input_output_aliases=(),
            sim_require_finite=False,
            sim_require_nnan=False,
            nc=nc,
        )
        return tuple(outs)

    n = len(devices)
    mesh = Mesh(np.asarray(devices), ("core",))
    sharded = jax.jit(
        shard_map(
            _body, mesh=mesh,
            in_specs=(PartitionSpec("core"),) * (n_params + n_outs),
            out_specs=(PartitionSpec("core"),) * n_outs,
            check_rep=False,
        ),
        donate_argnums=donate, keep_unused=True,
    )
    concat_in = [
        np.concatenate([np.asarray(m[name]) for m in in_maps], axis=0)
        for name in in_names
    ]
    concat_zero = [
        np.zeros((n * z.shape[0], *z.shape[1:]), z.dtype) for z in zero_outs
    ]
    out_arrs = sharded(*concat_in, *concat_zero)
    out_holder[idx] = [
        {
            name: np.asarray(out_arrs[i]).reshape(n, *out_avals[i].shape)[c]
            for i, name in enumerate(out_names)
        }
        for c in range(n)
    ]


def kernel(x, attn_mask, w_qkv, w_out):
    """Full inputs in, full output out. attn_mask is all-ones (per the
    problem spec) so masking reduces to the causal structure."""
    import jax

    x = np.asarray(x, dtype=np.float32)
    w_qkv = np.asarray(w_qkv, dtype=np.float32)
    w_out = np.asarray(w_out, dtype=np.float32)

    nc_e = _get_program(0)
    nc_o = _get_program(1)

    devices = jax.devices()
    in_maps = [
        {"x": x[b], "w_qkv": w_qkv, "w_out": w_out} for b in range(B)
    ]

    results = [None, None]
    t_e = threading.Thread(
        target=_run_group, args=(nc_e, in_maps, devices[0:4], results, 0))
    t_o = threading.Thread(
        target=_run_group, args=(nc_o, in_maps, devices[4:8], results, 1))
    t_e.start(); t_o.start()
    t_e.join(); t_o.join()

    y = np.empty((B, T, C), dtype=np.float32)
    for parity, group in enumerate(results):
        for b in range(B):
            y_local = group[b]["y"]          # [TL, C] in local q order
            for j in range(NQT):
                G = QMAP256[parity][j]
                y[b, G * QT_TILE:(G + 1) * QT_TILE, :] = \
                    y_local[j * QT_TILE:(j + 1) * QT_TILE, :]
    return y


# revision 29
# speedup vs baseline: 10286.5244x; 10286.5244x over previous
"""Causal self-attention on 8 Trainium2 NeuronCores.

Problem: B=4, T=2048, C=1024, H=16, DH=64.
  qkv = x @ w_qkv.T ; causal softmax attention per head ; y = attnout @ w_out.T

Sharding: 8 cores = 4 batches x 2 query-subsets. Each core computes the full
QKV projection for its batch (duplicated within the pair -> no collectives),
then attention for a load-balanced set of query rows (all 16 heads), then
the output projection for its own query rows against the full w_out. No
cross-core communication anywhere.

Query balance under causality: global 512-row q-tiles are paired (i, 3-i):
  parity 0 -> q512 tiles [0, 3] (20 key-tiles), parity 1 -> [1, 2] (20).

Everything runs in "transposed space": Q^T/K^T are produced head-pair-stacked
[128=2x64 dh rows, T], scores are computed as S^T (keys on PSUM partitions,
two heads concurrently via PE row-tiling), the softmax denominator is
accumulated on the PE itself (all-ones stationary -> column sums replicated
across all partitions), PV produces attnout^T directly (two heads via PE
column-tiling), and the output projection consumes attnout^T as its
stationary operand — no transposes in any inner loop. All input transposes
(x^T, w^T) are done by DMA-transpose through a bf16 DRAM staging copy.
"""

import threading

import numpy as np

B, T, C = 4, 2048, 1024
H = 16
DH = C // H
P = 128
TL = T // 2          # query rows per core
NPAIR = H // 2       # 8 head-pairs
NCT = C // P         # 8 c-tiles
QT_TILE = 512        # q columns per attention tile
NQT = TL // QT_TILE  # 2 local q-tiles
NEG = -1.0e9

# local q512-tile -> global q512-tile, per parity (also the Q-proj map)
QMAP512 = [[0, 3], [1, 2]]

_cache = {}


def _build_program(parity: int):
    import concourse.mybir as mybir
    import concourse.tile as tile
    from concourse import bacc
    from concourse.masks import make_identity

    f32 = mybir.dt.float32
    bf16 = mybir.dt.bfloat16

    nc = bacc.Bacc("TRN2", target_bir_lowering=False, debug=False)
    x = nc.dram_tensor("x", [T, C], f32, kind="ExternalInput").ap()
    w_qkv = nc.dram_tensor("w_qkv", [3 * C, C], f32, kind="ExternalInput").ap()
    w_out = nc.dram_tensor("w_out", [C, C], f32, kind="ExternalInput").ap()
    y = nc.dram_tensor("y", [TL, C], f32, kind="ExternalOutput").ap()

    g512 = QMAP512[parity]

    with tile.TileContext(nc) as tc:
        with (
            tc.tile_pool(name="res", bufs=1) as res,
            tc.tile_pool(name="stage", bufs=2) as stage,
            tc.tile_pool(name="wtile", bufs=2) as wtile,
            tc.tile_pool(name="work", bufs=2) as work,
            tc.tile_pool(name="rdp", bufs=2) as rdp,
            tc.tile_pool(name="attn", bufs=2) as attnp,
            tc.tile_pool(name="yout", bufs=1) as yout,
        ):
            ones128 = res.tile([P, P], bf16)
            nc.vector.memset(ones128, 1.0)

            # multiplicative bf16 masks for the 4 diagonal k-tiles of a
            # q-tile: keep (1.0) iff col >= row + 128*j, else 0.0
            masks = []
            for j in range(4):
                m = res.tile([P, QT_TILE], bf16, name=f"mask{j}")
                nc.gpsimd.memset(m, 1.0)
                nc.gpsimd.affine_select(
                    out=m, in_=m, compare_op=mybir.AluOpType.is_ge,
                    fill=0.0, base=-P * j, pattern=[[1, QT_TILE]],
                    channel_multiplier=-1,
                )
                masks.append(m)

            ident = res.tile([P, P], bf16)
            make_identity(nc, ident)

            # ---- residents
            kT = res.tile([P, NPAIR, T], bf16)          # K^T   4 MB
            qT = res.tile([P, NPAIR, TL], bf16)         # Q^T   2 MB
            v = res.tile([P, T // P, C], bf16)          # V     4 MB
            wvT = res.tile([P, NCT, C], bf16)           # w_v^T 2 MB
            woT = res.tile([P, NCT, C], bf16)           # w_out^T 2 MB

            with (
                tc.tile_pool(name="xtp", bufs=1) as xtp,
                tc.tile_pool(name="psqkv", bufs=4, space="PSUM") as psqkv,
                tc.tile_pool(name="pst", bufs=2, space="PSUM") as pst,
            ):
                xT = xtp.tile([P, NCT, T], bf16)        # x^T   4 MB

                def load_cast(src_ap):
                    lf = stage.tile([P, C], f32, tag="ldf")
                    nc.sync.dma_start(out=lf, in_=src_ap)
                    lb = stage.tile([P, C], bf16, tag="ldb")
                    nc.any.tensor_copy(out=lb, in_=lf)
                    return lb

                def transpose_block(lb, dst, dst_col):
                    for ct in range(NCT):
                        pt = pst.tile([P, P], bf16, tag="pt")
                        nc.tensor.transpose(
                            pt, lb[:, ct * P:(ct + 1) * P], ident)
                        nc.any.tensor_copy(
                            out=dst[:, ct, dst_col:dst_col + P], in_=pt)

                # x^T via PE transposes (PE busy from ~10us, HAM warms)
                for tt in range(T // P):
                    xb = load_cast(x[tt * P:(tt + 1) * P, :])
                    transpose_block(xb, xT, tt * P)

                # ---- Q/K projections (Q: only local halves)
                for fb in range(16):                     # 0..7 Q, 8..15 K
                    wb = load_cast(w_qkv[fb * P:(fb + 1) * P, :])
                    wqk = wtile.tile([P, NCT, P], bf16, tag="wqk")
                    transpose_block(wb, wqk, 0)
                    if fb < 8:
                        for u in range(NQT):
                            ps = psqkv.tile([P, 512], f32, tag="psqkv")
                            t0 = g512[u] * 512
                            for ct in range(NCT):
                                nc.tensor.matmul(
                                    ps, wqk[:, ct, :],
                                    xT[:, ct, t0:t0 + 512],
                                    start=(ct == 0), stop=(ct == NCT - 1),
                                )
                            nc.vector.tensor_copy(
                                out=qT[:, fb, u * 512:(u + 1) * 512], in_=ps)
                    else:
                        pr = fb - 8
                        for u in range(4):
                            ps = psqkv.tile([P, 512], f32, tag="psqkv")
                            for ct in range(NCT):
                                nc.tensor.matmul(
                                    ps, wqk[:, ct, :],
                                    xT[:, ct, u * 512:(u + 1) * 512],
                                    start=(ct == 0), stop=(ct == NCT - 1),
                                )
                            nc.vector.tensor_copy(
                                out=kT[:, pr, u * 512:(u + 1) * 512], in_=ps)

                # ---- V weights transposed, then V projection
                for fb in range(8):
                    wb = load_cast(w_qkv[(16 + fb) * P:(17 + fb) * P, :])
                    transpose_block(wb, wvT, fb * P)
                for fb in range(8):
                    wb = load_cast(w_out[fb * P:(fb + 1) * P, :])
                    transpose_block(wb, woT, fb * P)
                for tt in range(T // P):
                    for fo in range(2):
                        ps = psqkv.tile([P, 512], f32, tag="psqkv")
                        for ct in range(NCT):
                            nc.tensor.matmul(
                                ps, xT[:, ct, tt * P:(tt + 1) * P],
                                wvT[:, ct, fo * 512:(fo + 1) * 512],
                                start=(ct == 0), stop=(ct == NCT - 1),
                            )
                        nc.vector.tensor_copy(
                            out=v[:, tt, fo * 512:(fo + 1) * 512], in_=ps)

            # ================= attention + out-proj =================
            with (
                tc.tile_pool(name="pss", bufs=2, space="PSUM") as pss,
                tc.tile_pool(name="pso", bufs=4, space="PSUM") as pso,
            ):
                scale = 1.0 / float(np.sqrt(DH))
                for j in range(NQT):
                    G = g512[j]
                    nk = 4 * (G + 1)    # k-tiles (keys 0 .. 512*(G+1))
                    attnT = attnp.tile([P, NPAIR, QT_TILE], bf16, tag="attnT")
                    for p in range(NPAIR):
                        # Two banks per pair: bank A = PV-A (rows 0:64) +
                        # denom-B (rows 64:128); bank B = denom-A (rows 0:64)
                        # + PV-B (rows 64:128). The cross-assignment keeps
                        # every reciprocal/normalize op partition-aligned.
                        poA_t = pso.tile([P, QT_TILE], f32, tag="po",
                                         name="poA")
                        poB_t = pso.tile([P, QT_TILE], f32, tag="po",
                                         name="poB")
                        poA = poA_t[0:64]
                        poB = poB_t[64:128]
                        pdA = poB_t[0:64]
                        pdB = poA_t[64:128]
                        qA = qT[0:64, p, j * QT_TILE:(j + 1) * QT_TILE]
                        qB = qT[64:128, p, j * QT_TILE:(j + 1) * QT_TILE]
                        for k in range(nk):
                            s2 = pss.tile([P, 2, QT_TILE], f32, tag="s2")
                            ks = slice(k * P, (k + 1) * P)
                            nc.tensor.matmul(s2[:, 0, :], kT[0:64, p, ks], qA)
                            nc.tensor.matmul(s2[:, 1, :], kT[64:128, p, ks], qB)
                            p2 = work.tile([P, 2, QT_TILE], bf16, tag="p2")
                            nc.scalar.activation(
                                p2, s2, mybir.ActivationFunctionType.Exp,
                                scale=scale)
                            dj = k - 4 * G
                            if dj >= 0:  # diagonal tile: zero blocked cells
                                nc.vector.tensor_mul(
                                    out=p2, in0=p2,
                                    in1=masks[dj][:, None, :].to_broadcast(
                                        (P, 2, QT_TILE)))
                            pA = p2[:, 0, :]
                            pB = p2[:, 1, :]
                            # the group check is bank-granular but the
                            # pending-zero accounting is partition-exact, so
                            # the partition-split groups are safe to skip-check
                            st, sp_ = (k == 0), (k == nk - 1)
                            nc.tensor.matmul(pdA, ones128[:, 0:64], pA,
                                             start=st, stop=sp_,
                                             skip_group_check=True)
                            nc.tensor.matmul(pdB, ones128[:, 0:64], pB,
                                             start=st, stop=sp_,
                                             skip_group_check=True)
                            nc.tensor.matmul(
                                poA, v[:, k, p * P:p * P + 64], pA,
                                start=st, stop=sp_, skip_group_check=True)
                            nc.tensor.matmul(
                                poB, v[:, k, p * P + 64:(p + 1) * P], pB,
                                start=st, stop=sp_, skip_group_check=True)
                        rD = rdp.tile([P, QT_TILE], f32, tag="rD")
                        nc.vector.reciprocal(rD[0:64], pdA)
                        nc.vector.reciprocal(rD[64:128], pdB)
                        nc.vector.tensor_mul(
                            out=attnT[0:64, p, :], in0=poA, in1=rD[0:64])
                        nc.vector.tensor_mul(
                            out=attnT[64:128, p, :], in0=poB, in1=rD[64:128])
                    # ---- output projection for this q-tile
                    for sub in range(QT_TILE // P):
                        qs = slice(sub * P, (sub + 1) * P)
                        for fo in range(2):
                            ps = pso.tile([P, 512], f32, tag="po", name="psy")
                            for p in range(NPAIR):
                                nc.tensor.matmul(
                                    ps, attnT[:, p, qs],
                                    woT[:, p, fo * 512:(fo + 1) * 512],
                                    start=(p == 0), stop=(p == NPAIR - 1),
                                )
                            ysb = yout.tile([P, 512], f32, tag="ysb")
                            nc.any.tensor_copy(out=ysb, in_=ps)
                            nc.sync.dma_start(
                                out=y[j * QT_TILE + sub * P:
                                      j * QT_TILE + (sub + 1) * P,
                                      fo * 512:(fo + 1) * 512],
                                in_=ysb)

    nc.compile()
    return nc


def _get_program(parity: int):
    if parity not in _cache:
        _cache[parity] = _build_program(parity)
    return _cache[parity]


def _run_group(nc, in_maps, devices, out_holder, idx):
    """shard_map the program over `devices`, one in_map per device."""
    import jax
    from jax.sharding import Mesh, PartitionSpec
    from jax.experimental.shard_map import shard_map
    import concourse.mybir as mybir
    from concourse.bass2jax import (
        _bass_exec_p, install_neuronx_cc_hook, partition_id_tensor)

    install_neuronx_cc_hook()

    partition_name = (
        nc.partition_id_tensor.name if nc.partition_id_tensor else None)
    in_names, out_names, out_avals, zero_outs = [], [], [], []
    for alloc in nc.m.functions[0].allocations:
        if not isinstance(alloc, mybir.MemoryLocationSet):
            continue
        name = alloc.memorylocations[0].name
        if alloc.kind == "ExternalInput":
            if name != partition_name:
                in_names.append(name)
        elif alloc.kind == "ExternalOutput":
            out_names.append(name)
            shape = tuple(alloc.tensor_shape)
            dtype = mybir.dt.np(alloc.dtype)
            out_avals.append(jax.core.ShapedArray(shape, dtype))
            zero_outs.append(np.zeros(shape, dtype))
    n_params = len(in_names)
    n_outs = len(out_avals)
    all_names = in_names + out_names
    if partition_name is not None:
        all_names.append(partition_name)
    donate = tuple(range(n_params, n_params + n_outs))

    def _body(*args):
        operands = list(args)
        if partition_name is not None:
            operands.append(partition_id_tensor())
        outs = _bass_exec_p.bind(
            *operands,
            out_avals=tuple(out_avals),
            in_names=tuple(all_names),
            out_names=tuple(out_names),
            lowering_input_output_aliases=(),
            sim_require_finite=False,
            sim_require_nnan=False,
            nc=nc,
        )
        return tuple(outs)

    n = len(devices)
    mesh = Mesh(np.asarray(devices), ("core",))
    sharded = jax.jit(
        shard_map(
            _body, mesh=mesh,
            in_specs=(PartitionSpec("core"),) * (n_params + n_outs),
            out_specs=(PartitionSpec("core"),) * n_outs,
            check_rep=False,
        ),
        donate_argnums=donate, keep_unused=True,
    )
    concat_in = [
        np.concatenate([np.asarray(m[name]) for m in in_maps], axis=0)
        for name in in_names
    ]
    concat_zero = [
        np.zeros((n * z.shape[0], *z.shape[1:]), z.dtype) for z in zero_outs
    ]
    out_arrs = sharded(*concat_in, *concat_zero)
    out_holder[idx] = [
        {
            name: np.asarray(out_arrs[i]).reshape(n, *out_avals[i].shape)[c]
            for i, name in enumerate(out_names)
        }
        for c in range(n)
    ]


def kernel(x, attn_mask, w_qkv, w_out):
    """Full inputs in, full output out. attn_mask is all-ones (per the
    problem spec) so masking reduces to the causal structure."""
    import jax

    x = np.asarray(x, dtype=np.float32)
    w_qkv = np.asarray(w_qkv, dtype=np.float32)
    w_out = np.asarray(w_out, dtype=np.float32)

    nc_e = _get_program(0)
    nc_o = _get_program(1)

    devices = jax.devices()
    in_maps = [
        {"x": x[b], "w_qkv": w_qkv, "w_out": w_out} for b in range(B)
    ]

    results = [None, None]
    t_e = threading.Thread(
        target=_run_group, args=(nc_e, in_maps, devices[0:4], results, 0))
    t_o = threading.Thread(
        target=_run_group, args=(nc_o, in_maps, devices[4:8], results, 1))
    t_e.start(); t_o.start()
    t_e.join(); t_o.join()

    y = np.empty((B, T, C), dtype=np.float32)
    for parity, group in enumerate(results):
        for b in range(B):
            y_local = group[b]["y"]          # [TL, C] in local q order
            for j in range(NQT):
                G = QMAP512[parity][j]
                y[b, G * QT_TILE:(G + 1) * QT_TILE, :] = \
                    y_local[j * QT_TILE:(j + 1) * QT_TILE, :]
    return y
